# revision 60
# baseline (speedup 1.0000x reference)
"""CLSTMCell fused cell kernel for 8 Trainium2 NeuronCores.

Data-parallel over the batch: each of the 8 cores processes a 512-row batch
shard; weights are replicated.

The pre-activations have complex-multiplication structure. With
a = [x_r h_r], b = [x_i h_i]  (each [512, 2048]) and stacked weights
Wr = [R; Rr], Wi = [I; Ir]  (each [2048, 4096]):
    zr = a @ Wr + b @ Wi + br
    zi = b @ Wr - a @ Wi + bi
Karatsuba 3-product form (25% less tensor work than the 4-product form):
    q  = b @ Wi
    m1 = a @ Wr
    m3 = (a+b) @ (Wr-Wi)
    zr = m1 + q,  zi = m3 - m1 + q
All matmul operands are fp16 (10-bit mantissa; h max rel err ~2e-3 vs the
fp32 reference) with fp32 PSUM accumulation — full-rate on the PE and half
the HBM traffic of fp32. Per gate g (i,f,c,o): i,f,o -> hard_sigmoid,
c~ -> tanh, then c = f*c_prev + i*tanh(c~); h = o*tanh(c). The first U
output columns use zr's gates, the last U use zi's.

Device layout: output columns on PSUM partitions, batch on the free dim.
Work is organized in 32 groups (8 column-phases x 4 gates); each group
accumulates three dtype-uniform 16-step psum chains (q/m1/m3) from
[128k,128n] stationary weight tiles and [128k,512b] moving activation
blocks, then a short DVE/ACT combine drains the three banks into the gate
activations. s = a+b is computed on device from the chunked a/b tiles.
Weights stream per-group as six 256KB DMAs (2KB per partition line),
prefetched three groups ahead; groups 0-2's weights race the act stream
on the ACT engine's DGE queue so the first q chain starts as early as
possible.
"""

import sys

sys.path.insert(0, "/opt/trn_rl_repo")

import numpy as np

import concourse.bacc as bacc
import concourse.mybir as mybir
import concourse.tile as tile
from concourse.bass_utils import run_bass_kernel_spmd

N_CORES = 8
B, D, U = 4096, 1024, 1024
BS = B // N_CORES          # batch rows per core
P = 128                    # SBUF partitions
KB = (D + U) // P          # 16 contraction blocks of 128
NT = U // P                # 8 column-phases per gate
NGRP = NT * 4              # 32 (phase, gate) groups
QK = 2                     # act tiles span 2 k-blocks each
F32 = mybir.dt.float32
F16 = mybir.dt.float16
ADD = mybir.AluOpType.add
SUB = mybir.AluOpType.subtract
MULT = mybir.AluOpType.mult
MIN = mybir.AluOpType.min
WCOLS = 3 * KB * P         # weight dram cols per group row-block

_CACHE = {}


def _build():
    nc = bacc.Bacc("TRN2", target_bir_lowering=False, debug=False,
                   num_devices=N_CORES)
    Tanh = mybir.ActivationFunctionType.Tanh
    Relu = mybir.ActivationFunctionType.Relu

    din = {}
    din["aT"] = nc.dram_tensor("aT", [P, KB * BS], F16,
                               kind="ExternalInput").ap()
    din["bT"] = nc.dram_tensor("bT", [P, KB * BS], F16,
                               kind="ExternalInput").ap()
    din["w16"] = nc.dram_tensor("w16", [NGRP * P, WCOLS], F16,
                                kind="ExternalInput").ap()
    din["c_prevT"] = nc.dram_tensor("c_prevT", [2 * U, BS], F16,
                                    kind="ExternalInput").ap()
    din["brT"] = nc.dram_tensor("brT", [P, NGRP], F32,
                                kind="ExternalInput").ap()
    din["biT"] = nc.dram_tensor("biT", [P, NGRP], F32,
                                kind="ExternalInput").ap()
    h_outT = nc.dram_tensor("h_outT", [2 * U, BS], F16,
                            kind="ExternalOutput").ap()
    c_outT = nc.dram_tensor("c_outT", [2 * U, BS], F16,
                            kind="ExternalOutput").ap()

    with tile.TileContext(nc) as tc:
        with (
            tc.tile_pool(name="apool", bufs=KB // QK) as apool,
            tc.tile_pool(name="bpool", bufs=KB // QK) as bpool,
            tc.tile_pool(name="spool", bufs=KB // QK) as spool,
            tc.tile_pool(name="wp", bufs=24) as wp,
            tc.tile_pool(name="bias", bufs=4) as bias_p,
            tc.tile_pool(name="cprev", bufs=4) as cpool,
            tc.tile_pool(name="comb", bufs=8) as comb_p,
            tc.tile_pool(name="gat", bufs=12) as gat_p,
            tc.tile_pool(name="tmp", bufs=8) as tmp_p,
            tc.tile_pool(name="outs", bufs=6) as out_p,
            tc.tile_pool(name="psum", bufs=8, space="PSUM") as psum_p,
        ):
            QW = QK * BS  # columns per act chunk-tile
            HK = KB * P // 2  # weight cols per half-chain tile

            # --- weight prefetch: 3 chains x split tiles per group --------
            wtiles = {}

            def fetch_w(gi, eng, qsplit=2, parts="qas"):
                r0 = gi * P
                for ci, c in enumerate(("q", "a", "s")):
                    if c not in parts:
                        continue
                    base = ci * KB * P
                    ns = qsplit if c == "q" else 2
                    w = KB * P // ns
                    tiles = []
                    for i in range(ns):
                        ti = wp.tile([P, w], F16, tag="w",
                                     name=f"w{c}{i}_{gi}")
                        eng.dma_start(
                            ti[:],
                            din["w16"][r0:r0 + P,
                                       base + i * w:base + (i + 1) * w])
                        tiles.append(ti)
                    wtiles[(gi, c)] = tiles

            # --- resident activation chunks; s = a + b on device ----------
            a_q, b_q, s_q = [], [], []

            def dma_b(j):
                bt = bpool.tile([P, QW], F16, tag="b", name=f"b{j}")
                nc.sync.dma_start(bt[:], din["bT"][:, j * QW:(j + 1) * QW])
                b_q.append(bt)

            def dma_a(j):
                at = apool.tile([P, QW], F16, tag="a", name=f"a{j}")
                nc.gpsimd.dma_start(at[:],
                                    din["aT"][:, j * QW:(j + 1) * QW])
                a_q.append(at)

            # acts stream on the sync queue in first-use order (q chain
            # consumes b, then m1 consumes a); groups 0-2's weights race
            # concurrently on the ACT engine's queue
            for j in range(8):
                dma_b(j)
            for j in range(8):
                dma_a(j)
            fetch_w(0, nc.scalar, qsplit=4)
            fetch_w(1, nc.scalar)
            fetch_w(2, nc.scalar)

            def amov(k):
                return a_q[k // QK][:, (k % QK) * BS:(k % QK + 1) * BS]

            def bmov(k):
                return b_q[k // QK][:, (k % QK) * BS:(k % QK + 1) * BS]

            def smov(k):
                return s_q[k // QK][:, (k % QK) * BS:(k % QK + 1) * BS]

            for j in range(KB // QK):
                st = spool.tile([P, QW], F16, tag="s", name=f"s{j}")
                nc.vector.tensor_tensor(st[:], a_q[j][:], b_q[j][:], ADD)
                s_q.append(st)

            # --- per-partition bias tiles [128, 32]; col = g*8 + t --------
            braw, bhs = [], []

            def emit_bias():
                for name in ("brT", "biT"):
                    t = bias_p.tile([P, NGRP], F32, tag="bias",
                                    name=f"braw_{name}")
                    nc.scalar.dma_start(t[:], din[name][:, :])
                    braw.append(t)
                    t2 = bias_p.tile([P, NGRP], F32, tag="bias",
                                     name=f"bhs_{name}")
                    nc.vector.tensor_scalar(t2[:], t[:], 0.2, 0.5, MULT, ADD)
                    bhs.append(t2)

            # --- main loop: 8 phases x 4 gates -----------------------------
            for t in range(NT):
                cps = {}
                for z in range(2):
                    cp = cpool.tile([P, BS], F16, tag="cprev",
                                    name=f"cp_{t}_{z}")
                    rows0 = z * U + t * P
                    nc.gpsimd.dma_start(cp[:],
                                        din["c_prevT"][rows0:rows0 + P, :])
                    cps[z] = cp
                gacts = {}
                tc2s = {}
                for g in range(4):
                    gi = t * 4 + g
                    if gi + 3 < NGRP:
                        fetch_w(gi + 3, nc.sync)
                    m1 = psum_p.tile([P, BS], F32, tag="ps",
                                     name=f"m1_{gi}")
                    qp = psum_p.tile([P, BS], F32, tag="ps", name=f"q_{gi}")
                    m3 = None
                    if gi != NGRP - 1:
                        m3 = psum_p.tile([P, BS], F32, tag="ps",
                                         name=f"m3_{gi}")

                    def chain(ps, wkey, mov, gi=gi, bsl=None):
                        tiles = wtiles.get((gi, wkey))
                        span = KB // len(tiles)
                        for k in range(KB):
                            wt = tiles[k // span]
                            c0 = (k % span) * P
                            mv = mov(k) if bsl is None else mov(k)[:, bsl]
                            nc.tensor.matmul(
                                ps[:], wt[:, c0:c0 + P], mv,
                                start=(k == 0), stop=(k == KB - 1))

                    chain(qp, "q", bmov)
                    chain(m1, "a", amov)
                    m3hs = None
                    if gi == NGRP - 1:
                        # final group: m3 as two half-batch chains so the
                        # first half's drain overlaps the second half's
                        # matmuls
                        m3a = psum_p.tile([P, BS // 2], F32, tag="ps",
                                          name=f"m3a_{gi}")
                        m3b = psum_p.tile([P, BS // 2], F32, tag="ps",
                                          name=f"m3b_{gi}")
                        chain(m3a, "s", smov, bsl=slice(0, BS // 2))
                        chain(m3b, "s", smov, bsl=slice(BS // 2, BS))
                        m3hs = (m3a, m3b)
                    else:
                        chain(m3, "s", smov)
                    wtiles.pop((gi, "q"))
                    wtiles.pop((gi, "a"))
                    wtiles.pop((gi, "s"))
                    if not braw:
                        emit_bias()
                    # drain the three banks: zr = m1+q, zi = (m3+q)-m1
                    col = g * NT + t
                    qs = comb_p.tile([P, BS], F32, tag="comb",
                                     name=f"qs_{gi}")
                    nc.vector.tensor_copy(qs[:], qp[:])
                    zr = comb_p.tile([P, BS], F32, tag="comb",
                                     name=f"zr_{gi}")
                    nc.vector.tensor_tensor(zr[:], m1[:], qs[:], ADD)
                    if gi == NGRP - 1:
                        # kernel tail: z=0 drains full width while the m3
                        # chain still runs; z=1's whole drain (zi, o-gate,
                        # h, DMA) pipelines in half-batch chunks behind the
                        # final matmul
                        gt = gat_p.tile([P, BS], F32, tag="gat",
                                        name=f"g_{gi}_0")
                        nc.scalar.activation(
                            gt[:], zr[:], Relu,
                            bias=bhs[0][:, col:col + 1], scale=0.2)
                        hn = out_p.tile([P, BS], F16, tag="out",
                                        name=f"hn_{t}_0")
                        nc.vector.scalar_tensor_tensor(
                            hn[:], gt[:], 1.0, tc2s[0][:], MIN, MULT)
                        nc.gpsimd.dma_start(h_outT[t * P:t * P + P, :],
                                            hn[:])
                        rows1 = U + t * P
                        t0 = comb_p.tile([P, BS], F32, tag="comb",
                                         name=f"t0_{gi}")
                        zi = comb_p.tile([P, BS], F32, tag="comb",
                                         name=f"zi_{gi}")
                        ot = gat_p.tile([P, BS], F32, tag="gat",
                                        name=f"g_{gi}_1")
                        hn1 = out_p.tile([P, BS], F16, tag="out",
                                         name=f"hn_{t}_1")
                        for hi, h0 in enumerate((0, BS // 2)):
                            sl = slice(h0, h0 + BS // 2)
                            nc.vector.tensor_tensor(
                                t0[:, sl], m3hs[hi][:], qs[:, sl], ADD)
                            nc.vector.tensor_tensor(
                                zi[:, sl], t0[:, sl], m1[:, sl], SUB)
                            nc.scalar.activation(
                                ot[:, sl], zi[:, sl], Relu,
                                bias=bhs[1][:, col:col + 1], scale=0.2)
                            nc.vector.scalar_tensor_tensor(
                                hn1[:, sl], ot[:, sl], 1.0,
                                tc2s[1][:, sl], MIN, MULT)
                            nc.gpsimd.dma_start(
                                h_outT[rows1:rows1 + P, sl], hn1[:, sl])
                        continue
                    t0 = comb_p.tile([P, BS], F32, tag="comb",
                                     name=f"t0_{gi}")
                    nc.vector.tensor_tensor(t0[:], m3[:], qs[:], ADD)
                    zi = comb_p.tile([P, BS], F32, tag="comb",
                                     name=f"zi_{gi}")
                    nc.vector.tensor_tensor(zi[:], t0[:], m1[:], SUB)
                    for z, zz in enumerate((zr, zi)):
                        gt = gat_p.tile([P, BS], F32, tag="gat",
                                        name=f"g_{gi}_{z}")
                        if g == 2:
                            nc.scalar.activation(
                                gt[:], zz[:], Tanh,
                                bias=braw[z][:, col:col + 1], scale=1.0)
                        else:
                            # relu(0.2*z + 0.2*b + 0.5); min(.,1) rides the
                            # consuming DVE op
                            nc.scalar.activation(
                                gt[:], zz[:], Relu,
                                bias=bhs[z][:, col:col + 1], scale=0.2)
                        gacts[(g, z)] = gt
                    if g == 2:
                        for z in range(2):
                            rows0 = z * U + t * P
                            t1 = tmp_p.tile([P, BS], F32, tag="tmp",
                                            name=f"t1_{t}_{z}")
                            nc.vector.scalar_tensor_tensor(
                                t1[:], gacts[(1, z)][:], 1.0, cps[z][:],
                                MIN, MULT)
                            t2 = tmp_p.tile([P, BS], F32, tag="tmp",
                                            name=f"t2_{t}_{z}")
                            nc.vector.scalar_tensor_tensor(
                                t2[:], gacts[(0, z)][:], 1.0,
                                gacts[(2, z)][:], MIN, MULT)
                            cn = out_p.tile([P, BS], F16, tag="out",
                                            name=f"cn_{t}_{z}")
                            nc.vector.tensor_tensor(cn[:], t1[:], t2[:], ADD)
                            nc.gpsimd.dma_start(
                                c_outT[rows0:rows0 + P, :], cn[:])
                            tc2 = tmp_p.tile([P, BS], F32, tag="tmp",
                                             name=f"tc2_{t}_{z}")
                            nc.scalar.activation(tc2[:], cn[:], Tanh)
                            tc2s[z] = tc2
                    if g == 3:
                        for z in range(2):
                            rows0 = z * U + t * P
                            hn = out_p.tile([P, BS], F16, tag="out",
                                            name=f"hn_{t}_{z}")
                            nc.vector.scalar_tensor_tensor(
                                hn[:], gacts[(3, z)][:], 1.0, tc2s[z][:],
                                MIN, MULT)
                            nc.gpsimd.dma_start(
                                h_outT[rows0:rows0 + P, :], hn[:])

    nc.compile()
    return nc


def _in_maps(inputs, h_tm1, c_tm1, wr, wi, wrr, wir, br, bi):
    Wr = np.vstack([wr, wrr])            # [2048, 4096]
    Wi = np.vstack([wi, wir])
    Wd = Wr - Wi

    def perm(W):  # [2048, 4096] -> [NGRP*P rows, KB*P cols] fp16
        v = W.reshape(KB, P, 4, NT, P).transpose(3, 2, 1, 0, 4)
        return v.reshape(NGRP * P, KB * P)

    w16 = np.ascontiguousarray(
        np.concatenate([perm(Wi), perm(Wr), perm(Wd)], axis=1),
        dtype=np.float16)
    brT = np.ascontiguousarray(
        br.reshape(4, NT, P).transpose(2, 0, 1).reshape(P, NGRP))
    biT = np.ascontiguousarray(
        bi.reshape(4, NT, P).transpose(2, 0, 1).reshape(P, NGRP))

    def actperm(m):  # [512, 2048] -> [128, 16*512] fp16, part = k-part
        v = m.T.reshape(KB, P, BS).transpose(1, 0, 2).reshape(P, KB * BS)
        return np.ascontiguousarray(v, dtype=np.float16)

    maps = []
    for c in range(N_CORES):
        rows = slice(c * BS, (c + 1) * BS)
        a = np.hstack([inputs[rows, :D], h_tm1[rows, :U]])
        b = np.hstack([inputs[rows, D:], h_tm1[rows, U:]])
        maps.append({
            "aT": actperm(a),
            "bT": actperm(b),
            "c_prevT": np.ascontiguousarray(
                c_tm1[rows].T, dtype=np.float16),
            "w16": w16,
            "brT": brT, "biT": biT,
        })
    return maps


def kernel(inputs, h_tm1, c_tm1, real_kernel, imaginary_kernel,
           real_recurrent_kernel, imaginary_recurrent_kernel,
           real_bias, imaginary_bias):
    if "nc" not in _CACHE:
        _CACHE["nc"] = _build()
    nc = _CACHE["nc"]

    maps = _in_maps(
        np.ascontiguousarray(inputs, dtype=np.float32),
        np.ascontiguousarray(h_tm1, dtype=np.float32),
        np.ascontiguousarray(c_tm1, dtype=np.float32),
        np.ascontiguousarray(real_kernel, dtype=np.float32),
        np.ascontiguousarray(imaginary_kernel, dtype=np.float32),
        np.ascontiguousarray(real_recurrent_kernel, dtype=np.float32),
        np.ascontiguousarray(imaginary_recurrent_kernel, dtype=np.float32),
        np.ascontiguousarray(real_bias, dtype=np.float32),
        np.ascontiguousarray(imaginary_bias, dtype=np.float32),
    )
    res = run_bass_kernel_spmd(nc, maps, list(range(N_CORES)))
    h = np.concatenate(
        [res.results[c]["h_outT"].T.astype(np.float32) for c in range(N_CORES)], axis=0)
    c = np.concatenate(
        [res.results[c]["c_outT"].T.astype(np.float32) for c in range(N_CORES)], axis=0)
    return np.ascontiguousarray(h), np.ascontiguousarray(c)


# revision 61
# speedup vs baseline: 1.0085x; 1.0085x over previous
"""CLSTMCell fused cell kernel for 8 Trainium2 NeuronCores.

Data-parallel over the batch: each of the 8 cores processes a 512-row batch
shard; weights are replicated.

The pre-activations have complex-multiplication structure. With
a = [x_r h_r], b = [x_i h_i]  (each [512, 2048]) and stacked weights
Wr = [R; Rr], Wi = [I; Ir]  (each [2048, 4096]):
    zr = a @ Wr + b @ Wi + br
    zi = b @ Wr - a @ Wi + bi
Karatsuba 3-product form (25% less tensor work than the 4-product form):
    q  = b @ Wi
    m1 = a @ Wr
    m3 = (a+b) @ (Wr-Wi)
    zr = m1 + q,  zi = m3 - m1 + q
All matmul operands are fp16 (10-bit mantissa; h max rel err ~2e-3 vs the
fp32 reference) with fp32 PSUM accumulation — full-rate on the PE and half
the HBM traffic of fp32. Per gate g (i,f,c,o): i,f,o -> hard_sigmoid,
c~ -> tanh, then c = f*c_prev + i*tanh(c~); h = o*tanh(c). The first U
output columns use zr's gates, the last U use zi's.

Device layout: output columns on PSUM partitions, batch on the free dim.
Work is organized in 32 groups (8 column-phases x 4 gates); each group
accumulates three dtype-uniform 16-step psum chains (q/m1/m3) from
[128k,128n] stationary weight tiles and [128k,512b] moving activation
blocks, then a short DVE/ACT combine drains the three banks into the gate
activations. s = a+b is computed on device from the chunked a/b tiles.
Weights stream per-group as six 256KB DMAs (2KB per partition line),
prefetched three groups ahead; groups 0-2's weights race the act stream
on the ACT engine's DGE queue so the first q chain starts as early as
possible.
"""

import sys

sys.path.insert(0, "/opt/trn_rl_repo")

import numpy as np

import concourse.bacc as bacc
import concourse.mybir as mybir
import concourse.tile as tile
from concourse.bass_utils import run_bass_kernel_spmd

N_CORES = 8
B, D, U = 4096, 1024, 1024
BS = B // N_CORES          # batch rows per core
P = 128                    # SBUF partitions
KB = (D + U) // P          # 16 contraction blocks of 128
NT = U // P                # 8 column-phases per gate
NGRP = NT * 4              # 32 (phase, gate) groups
QK = 2                     # act tiles span 2 k-blocks each
F32 = mybir.dt.float32
F16 = mybir.dt.float16
ADD = mybir.AluOpType.add
SUB = mybir.AluOpType.subtract
MULT = mybir.AluOpType.mult
MIN = mybir.AluOpType.min
WCOLS = 3 * KB * P         # weight dram cols per group row-block

_CACHE = {}


def _build():
    nc = bacc.Bacc("TRN2", target_bir_lowering=False, debug=False,
                   num_devices=N_CORES)
    Tanh = mybir.ActivationFunctionType.Tanh
    Relu = mybir.ActivationFunctionType.Relu

    din = {}
    din["aT"] = nc.dram_tensor("aT", [P, KB * BS], F16,
                               kind="ExternalInput").ap()
    din["bT"] = nc.dram_tensor("bT", [P, KB * BS], F16,
                               kind="ExternalInput").ap()
    din["w16"] = nc.dram_tensor("w16", [NGRP * P, WCOLS], F16,
                                kind="ExternalInput").ap()
    din["c_prevT"] = nc.dram_tensor("c_prevT", [2 * U, BS], F16,
                                    kind="ExternalInput").ap()
    din["brT"] = nc.dram_tensor("brT", [P, NGRP], F32,
                                kind="ExternalInput").ap()
    din["biT"] = nc.dram_tensor("biT", [P, NGRP], F32,
                                kind="ExternalInput").ap()
    h_outT = nc.dram_tensor("h_outT", [2 * U, BS], F16,
                            kind="ExternalOutput").ap()
    c_outT = nc.dram_tensor("c_outT", [2 * U, BS], F16,
                            kind="ExternalOutput").ap()

    with tile.TileContext(nc) as tc:
        with (
            tc.tile_pool(name="apool", bufs=KB // QK) as apool,
            tc.tile_pool(name="bpool", bufs=KB // QK) as bpool,
            tc.tile_pool(name="spool", bufs=KB // QK) as spool,
            tc.tile_pool(name="wp", bufs=24) as wp,
            tc.tile_pool(name="bias", bufs=4) as bias_p,
            tc.tile_pool(name="cprev", bufs=4) as cpool,
            tc.tile_pool(name="comb", bufs=8) as comb_p,
            tc.tile_pool(name="gat", bufs=12) as gat_p,
            tc.tile_pool(name="tmp", bufs=8) as tmp_p,
            tc.tile_pool(name="outs", bufs=6) as out_p,
            tc.tile_pool(name="psum", bufs=8, space="PSUM") as psum_p,
        ):
            QW = QK * BS  # columns per act chunk-tile
            HK = KB * P // 2  # weight cols per half-chain tile

            # --- weight prefetch: 3 chains x split tiles per group --------
            wtiles = {}

            def fetch_w(gi, eng, qsplit=2, parts="qas"):
                r0 = gi * P
                for ci, c in enumerate(("q", "a", "s")):
                    if c not in parts:
                        continue
                    base = ci * KB * P
                    ns = qsplit if c == "q" else 2
                    w = KB * P // ns
                    tiles = []
                    for i in range(ns):
                        ti = wp.tile([P, w], F16, tag="w",
                                     name=f"w{c}{i}_{gi}")
                        eng.dma_start(
                            ti[:],
                            din["w16"][r0:r0 + P,
                                       base + i * w:base + (i + 1) * w])
                        tiles.append(ti)
                    wtiles[(gi, c)] = tiles

            # --- resident activation chunks; s = a + b on device ----------
            a_q, b_q, s_q = [], [], []

            def dma_b(j):
                bt = bpool.tile([P, QW], F16, tag="b", name=f"b{j}")
                nc.sync.dma_start(bt[:], din["bT"][:, j * QW:(j + 1) * QW])
                b_q.append(bt)

            def dma_a(j):
                # a0-3 ride Pool's free queue (m1 needs them first);
                # a4-7 follow b on sync, landing just before m1's k8-15
                at = apool.tile([P, QW], F16, tag="a", name=f"a{j}")
                eng = nc.gpsimd if j < 4 else nc.sync
                eng.dma_start(at[:], din["aT"][:, j * QW:(j + 1) * QW])
                a_q.append(at)

            # acts stream on the sync queue in first-use order (q chain
            # consumes b, then m1 consumes a); groups 0-2's weights race
            # concurrently on the ACT engine's queue
            for j in range(8):
                dma_b(j)
            for j in range(8):
                dma_a(j)
            fetch_w(0, nc.scalar, qsplit=4)
            fetch_w(1, nc.scalar)
            fetch_w(2, nc.scalar)

            def amov(k):
                return a_q[k // QK][:, (k % QK) * BS:(k % QK + 1) * BS]

            def bmov(k):
                return b_q[k // QK][:, (k % QK) * BS:(k % QK + 1) * BS]

            def smov(k):
                return s_q[k // QK][:, (k % QK) * BS:(k % QK + 1) * BS]

            for j in range(KB // QK):
                st = spool.tile([P, QW], F16, tag="s", name=f"s{j}")
                nc.vector.tensor_tensor(st[:], a_q[j][:], b_q[j][:], ADD)
                s_q.append(st)

            # --- per-partition bias tiles [128, 32]; col = g*8 + t --------
            braw, bhs = [], []

            def emit_bias():
                for name in ("brT", "biT"):
                    t = bias_p.tile([P, NGRP], F32, tag="bias",
                                    name=f"braw_{name}")
                    nc.scalar.dma_start(t[:], din[name][:, :])
                    braw.append(t)
                    t2 = bias_p.tile([P, NGRP], F32, tag="bias",
                                     name=f"bhs_{name}")
                    nc.vector.tensor_scalar(t2[:], t[:], 0.2, 0.5, MULT, ADD)
                    bhs.append(t2)

            # --- main loop: 8 phases x 4 gates -----------------------------
            for t in range(NT):
                cps = {}
                for z in range(2):
                    cp = cpool.tile([P, BS], F16, tag="cprev",
                                    name=f"cp_{t}_{z}")
                    rows0 = z * U + t * P
                    nc.gpsimd.dma_start(cp[:],
                                        din["c_prevT"][rows0:rows0 + P, :])
                    cps[z] = cp
                gacts = {}
                tc2s = {}
                for g in range(4):
                    gi = t * 4 + g
                    if gi + 3 < NGRP:
                        fetch_w(gi + 3, nc.sync)
                    m1 = psum_p.tile([P, BS], F32, tag="ps",
                                     name=f"m1_{gi}")
                    qp = psum_p.tile([P, BS], F32, tag="ps", name=f"q_{gi}")
                    m3 = None
                    if gi != NGRP - 1:
                        m3 = psum_p.tile([P, BS], F32, tag="ps",
                                         name=f"m3_{gi}")

                    def chain(ps, wkey, mov, gi=gi, bsl=None):
                        tiles = wtiles.get((gi, wkey))
                        span = KB // len(tiles)
                        for k in range(KB):
                            wt = tiles[k // span]
                            c0 = (k % span) * P
                            mv = mov(k) if bsl is None else mov(k)[:, bsl]
                            nc.tensor.matmul(
                                ps[:], wt[:, c0:c0 + P], mv,
                                start=(k == 0), stop=(k == KB - 1))

                    chain(qp, "q", bmov)
                    chain(m1, "a", amov)
                    m3hs = None
                    if gi == NGRP - 1:
                        # final group: m3 as two half-batch chains so the
                        # first half's drain overlaps the second half's
                        # matmuls
                        m3a = psum_p.tile([P, BS // 2], F32, tag="ps",
                                          name=f"m3a_{gi}")
                        m3b = psum_p.tile([P, BS // 2], F32, tag="ps",
                                          name=f"m3b_{gi}")
                        chain(m3a, "s", smov, bsl=slice(0, BS // 2))
                        chain(m3b, "s", smov, bsl=slice(BS // 2, BS))
                        m3hs = (m3a, m3b)
                    else:
                        chain(m3, "s", smov)
                    wtiles.pop((gi, "q"))
                    wtiles.pop((gi, "a"))
                    wtiles.pop((gi, "s"))
                    if not braw:
                        emit_bias()
                    # drain the three banks: zr = m1+q, zi = (m3+q)-m1
                    col = g * NT + t
                    qs = comb_p.tile([P, BS], F32, tag="comb",
                                     name=f"qs_{gi}")
                    nc.vector.tensor_copy(qs[:], qp[:])
                    zr = comb_p.tile([P, BS], F32, tag="comb",
                                     name=f"zr_{gi}")
                    nc.vector.tensor_tensor(zr[:], m1[:], qs[:], ADD)
                    if gi == NGRP - 1:
                        # kernel tail: z=0 drains full width while the m3
                        # chain still runs; z=1's whole drain (zi, o-gate,
                        # h, DMA) pipelines in half-batch chunks behind the
                        # final matmul
                        gt = gat_p.tile([P, BS], F32, tag="gat",
                                        name=f"g_{gi}_0")
                        nc.scalar.activation(
                            gt[:], zr[:], Relu,
                            bias=bhs[0][:, col:col + 1], scale=0.2)
                        hn = out_p.tile([P, BS], F16, tag="out",
                                        name=f"hn_{t}_0")
                        nc.vector.scalar_tensor_tensor(
                            hn[:], gt[:], 1.0, tc2s[0][:], MIN, MULT)
                        nc.gpsimd.dma_start(h_outT[t * P:t * P + P, :],
                                            hn[:])
                        rows1 = U + t * P
                        t0 = comb_p.tile([P, BS], F32, tag="comb",
                                         name=f"t0_{gi}")
                        zi = comb_p.tile([P, BS], F32, tag="comb",
                                         name=f"zi_{gi}")
                        ot = gat_p.tile([P, BS], F32, tag="gat",
                                        name=f"g_{gi}_1")
                        hn1 = out_p.tile([P, BS], F16, tag="out",
                                         name=f"hn_{t}_1")
                        for hi, h0 in enumerate((0, BS // 2)):
                            sl = slice(h0, h0 + BS // 2)
                            nc.vector.tensor_tensor(
                                t0[:, sl], m3hs[hi][:], qs[:, sl], ADD)
                            nc.vector.tensor_tensor(
                                zi[:, sl], t0[:, sl], m1[:, sl], SUB)
                            nc.scalar.activation(
                                ot[:, sl], zi[:, sl], Relu,
                                bias=bhs[1][:, col:col + 1], scale=0.2)
                            nc.vector.scalar_tensor_tensor(
                                hn1[:, sl], ot[:, sl], 1.0,
                                tc2s[1][:, sl], MIN, MULT)
                            nc.gpsimd.dma_start(
                                h_outT[rows1:rows1 + P, sl], hn1[:, sl])
                        continue
                    t0 = comb_p.tile([P, BS], F32, tag="comb",
                                     name=f"t0_{gi}")
                    nc.vector.tensor_tensor(t0[:], m3[:], qs[:], ADD)
                    zi = comb_p.tile([P, BS], F32, tag="comb",
                                     name=f"zi_{gi}")
                    nc.vector.tensor_tensor(zi[:], t0[:], m1[:], SUB)
                    for z, zz in enumerate((zr, zi)):
                        gt = gat_p.tile([P, BS], F32, tag="gat",
                                        name=f"g_{gi}_{z}")
                        if g == 2:
                            nc.scalar.activation(
                                gt[:], zz[:], Tanh,
                                bias=braw[z][:, col:col + 1], scale=1.0)
                        else:
                            # relu(0.2*z + 0.2*b + 0.5); min(.,1) rides the
                            # consuming DVE op
                            nc.scalar.activation(
                                gt[:], zz[:], Relu,
                                bias=bhs[z][:, col:col + 1], scale=0.2)
                        gacts[(g, z)] = gt
                    if g == 2:
                        for z in range(2):
                            rows0 = z * U + t * P
                            t1 = tmp_p.tile([P, BS], F32, tag="tmp",
                                            name=f"t1_{t}_{z}")
                            nc.vector.scalar_tensor_tensor(
                                t1[:], gacts[(1, z)][:], 1.0, cps[z][:],
                                MIN, MULT)
                            t2 = tmp_p.tile([P, BS], F32, tag="tmp",
                                            name=f"t2_{t}_{z}")
                            nc.vector.scalar_tensor_tensor(
                                t2[:], gacts[(0, z)][:], 1.0,
                                gacts[(2, z)][:], MIN, MULT)
                            cn = out_p.tile([P, BS], F16, tag="out",
                                            name=f"cn_{t}_{z}")
                            nc.vector.tensor_tensor(cn[:], t1[:], t2[:], ADD)
                            nc.gpsimd.dma_start(
                                c_outT[rows0:rows0 + P, :], cn[:])
                            tc2 = tmp_p.tile([P, BS], F32, tag="tmp",
                                             name=f"tc2_{t}_{z}")
                            nc.scalar.activation(tc2[:], cn[:], Tanh)
                            tc2s[z] = tc2
                    if g == 3:
                        for z in range(2):
                            rows0 = z * U + t * P
                            hn = out_p.tile([P, BS], F16, tag="out",
                                            name=f"hn_{t}_{z}")
                            nc.vector.scalar_tensor_tensor(
                                hn[:], gacts[(3, z)][:], 1.0, tc2s[z][:],
                                MIN, MULT)
                            nc.gpsimd.dma_start(
                                h_outT[rows0:rows0 + P, :], hn[:])

    nc.compile()
    return nc


def _in_maps(inputs, h_tm1, c_tm1, wr, wi, wrr, wir, br, bi):
    Wr = np.vstack([wr, wrr])            # [2048, 4096]
    Wi = np.vstack([wi, wir])
    Wd = Wr - Wi

    def perm(W):  # [2048, 4096] -> [NGRP*P rows, KB*P cols] fp16
        v = W.reshape(KB, P, 4, NT, P).transpose(3, 2, 1, 0, 4)
        return v.reshape(NGRP * P, KB * P)

    w16 = np.ascontiguousarray(
        np.concatenate([perm(Wi), perm(Wr), perm(Wd)], axis=1),
        dtype=np.float16)
    brT = np.ascontiguousarray(
        br.reshape(4, NT, P).transpose(2, 0, 1).reshape(P, NGRP))
    biT = np.ascontiguousarray(
        bi.reshape(4, NT, P).transpose(2, 0, 1).reshape(P, NGRP))

    def actperm(m):  # [512, 2048] -> [128, 16*512] fp16, part = k-part
        v = m.T.reshape(KB, P, BS).transpose(1, 0, 2).reshape(P, KB * BS)
        return np.ascontiguousarray(v, dtype=np.float16)

    maps = []
    for c in range(N_CORES):
        rows = slice(c * BS, (c + 1) * BS)
        a = np.hstack([inputs[rows, :D], h_tm1[rows, :U]])
        b = np.hstack([inputs[rows, D:], h_tm1[rows, U:]])
        maps.append({
            "aT": actperm(a),
            "bT": actperm(b),
            "c_prevT": np.ascontiguousarray(
                c_tm1[rows].T, dtype=np.float16),
            "w16": w16,
            "brT": brT, "biT": biT,
        })
    return maps


def kernel(inputs, h_tm1, c_tm1, real_kernel, imaginary_kernel,
           real_recurrent_kernel, imaginary_recurrent_kernel,
           real_bias, imaginary_bias):
    if "nc" not in _CACHE:
        _CACHE["nc"] = _build()
    nc = _CACHE["nc"]

    maps = _in_maps(
        np.ascontiguousarray(inputs, dtype=np.float32),
        np.ascontiguousarray(h_tm1, dtype=np.float32),
        np.ascontiguousarray(c_tm1, dtype=np.float32),
        np.ascontiguousarray(real_kernel, dtype=np.float32),
        np.ascontiguousarray(imaginary_kernel, dtype=np.float32),
        np.ascontiguousarray(real_recurrent_kernel, dtype=np.float32),
        np.ascontiguousarray(imaginary_recurrent_kernel, dtype=np.float32),
        np.ascontiguousarray(real_bias, dtype=np.float32),
        np.ascontiguousarray(imaginary_bias, dtype=np.float32),
    )
    res = run_bass_kernel_spmd(nc, maps, list(range(N_CORES)))
    h = np.concatenate(
        [res.results[c]["h_outT"].T.astype(np.float32) for c in range(N_CORES)], axis=0)
    c = np.concatenate(
        [res.results[c]["c_outT"].T.astype(np.float32) for c in range(N_CORES)], axis=0)
    return np.ascontiguousarray(h), np.ascontiguousarray(c)
